# revision 21
# baseline (speedup 1.0000x reference)
"""DSAttention Trainium2 kernel (8 NeuronCores, SPMD).

Sharding: batch (B=2) x head-groups (4 heads each) -> 8 cores.
Core c handles batch b=c//4, heads 4*(c%4) .. 4*(c%4)+3.

Per-core math (feature-major "transposed" layouts so softmax bias/scale land
on partition axes):
  q_t = Wq_p @ hs_b.T          [256, 2048]   (+bq per-partition)
  k_t = Wk_p @ hs_b.T          [256, 2048]   (+bk per-partition)
  v   = hs_b @ Wv_p.T          [2048, 256]   (per k-tile, with a ones column
                                              per head -> softmax denominator)
Phase B is organized as 16 units = (q-quarter 0..3, head-pair 0..1); each
unit's k-tile loop does, per kt:
  scores: head A -> psS[:, 0:512], head B -> psS[:, 512:1024]  (K=64 matmuls
          on PE row-groups 0-63 / 64-127, auto tile_position)
  e = exp(psS * tau/8 + delta_k/8)  -- ONE [128,1024] fused ACT exp per kt;
          ACT is the bottleneck engine (~1.2us/iter), everything else hides
  ctx_h[65, 512] += [v_h | 1].T @ e_half     accumulated over 16 k-tiles
Then ctx[0:64] /= ctx[64] (DRAM-bounce broadcast + DVE reciprocal/mul).
Out-projection chunks are interleaved into later units' k-loops (their dense
K=128 matmuls double as HAM keepalive); a dedicated filler accumulator covers
units that have no out-proj work yet (K=64 scores matmuls do not register in
the HAM activity monitor, and ctx M=65 alone sits below the throttle-hold
threshold -> the whole phase would pin at 1.2 GHz without them).
Host: out[b] = sum of the 4 head-group partials + bv @ Wo.T + bo
(softmax rows sum to 1, so the v/out biases commute to the host exactly).

All matmuls in float32r (~1.2e-4 input rounding, full PE rate at N>=256).
"""

import sys

for _p in ("/opt/trn_rl_repo", "/opt/pypackages"):
    if _p not in sys.path:
        sys.path.append(_p)

import numpy as np

import concourse.bass as bass
import concourse.tile as tile
from concourse import bacc, mybir
from concourse.bass_utils import run_bass_kernel_spmd

B, L, H = 2, 2048, 1024
NH, HD = 16, 64
NCORES = 8
HPC = 4  # heads per core
FPC = HPC * HD  # 256
NKT = L // 128  # 16 k-tiles
NHC = H // 128  # 8 H-contraction chunks

F32 = mybir.dt.float32
F32R = mybir.dt.float32r

_NC_CACHE = {}

# Dedup consecutive identical LDWEIGHTS in walrus codegen: every fp32r matmul
# self-loads its stationary operand, and consecutive matmuls often share it.
import concourse.bass_utils as _bu

_orig_run_command = _bu.run_command


def _run_command_ldwopt(cmd, *a, **kw):
    if isinstance(cmd, list):
        cmd = [
            "--enable-ldw-opt=true" if c == "--enable-ldw-opt=false" else c
            for c in cmd
        ]
    return _orig_run_command(cmd, *a, **kw)


_bu.run_command = _run_command_ldwopt


def _build_kernel():
    nc = bacc.Bacc(None, target_bir_lowering=False, debug=False)

    hs_t = nc.declare_dram_parameter("hs_t", [H, L], F32, isOutput=False)
    wq_t = nc.declare_dram_parameter("wq_t", [H, FPC], F32, isOutput=False)
    wk_t = nc.declare_dram_parameter("wk_t", [H, FPC], F32, isOutput=False)
    wv_t = nc.declare_dram_parameter("wv_t", [H, FPC], F32, isOutput=False)
    wo_t = nc.declare_dram_parameter("wo_t", [FPC, H], F32, isOutput=False)
    bq2 = nc.declare_dram_parameter("bq2", [128, 2], F32, isOutput=False)
    bk2 = nc.declare_dram_parameter("bk2", [128, 2], F32, isOutput=False)
    tau8 = nc.declare_dram_parameter("tau8", [128, 1], F32, isOutput=False)
    delta8 = nc.declare_dram_parameter("delta8", [128, NKT], F32, isOutput=False)
    outs_d = [
        nc.declare_dram_parameter(f"out{hp}", [L, H], F32, isOutput=True)
        for hp in range(2)
    ]

    with tile.TileContext(nc) as tc:
        with (
            tc.tile_pool(name="persist", bufs=1) as persist,
            tc.tile_pool(name="hsw", bufs=1) as hsw,
            # PSUM (8 banks): scores [128,1024] x2 (4) + ctx [65,512] x2 (2)
            # + out-proj [128,512] x1 (1) + filler [65,512] x1 (1)
            tc.tile_pool(name="sc_ps", bufs=2, space="PSUM") as sc_ps,
            tc.tile_pool(name="ctx_ps", bufs=2, space="PSUM") as ctx_ps,
            tc.tile_pool(name="work", bufs=4) as work,
            tc.tile_pool(name="dscratch", bufs=2, space="DRAM") as dscratch,
        ):
            # ---- input loads: per-chunk interleave across both HWDGE queues
            # (hs_c on one, wq/wk/wv_c on the other, alternating) so the two
            # rings split the 12MB roughly evenly and chunk-set c lands ~5us
            # after c-1 -> Q/K projection streams behind the DMA.
            hs_sb = []
            w_sb = {"q": [], "k": [], "v": []}
            queues = [nc.sync, nc.scalar]
            for c in range(NHC):
                qa, qb = queues[c % 2], queues[(c + 1) % 2]
                t = hsw.tile([128, L], F32R, tag=f"hs{c}", name=f"hs{c}")
                qa.dma_start(out=t[:], in_=hs_t[c * 128 : (c + 1) * 128, :].bitcast(F32R))
                hs_sb.append(t)
                for name, w in (("q", wq_t), ("k", wk_t), ("v", wv_t)):
                    wt = hsw.tile([128, FPC], F32R, tag=f"w{name}{c}", name=f"w{name}{c}")
                    if name == "v":
                        qb.dma_start(out=wt[:], in_=w[c * 128 : (c + 1) * 128, :].bitcast(F32R))
                    else:
                        # hp0 columns now; hp1 columns deferred below
                        qb.dma_start(
                            out=wt[:, 0:128],
                            in_=w[c * 128 : (c + 1) * 128, 0:128].bitcast(F32R),
                        )
                    w_sb[name].append(wt)
                if c == 0:
                    bq_sb = persist.tile([128, 2], F32, tag="bq")
                    qa.dma_start(out=bq_sb[:], in_=bq2[:])
                    bk_sb = persist.tile([128, 2], F32, tag="bk")
                    qa.dma_start(out=bk_sb[:], in_=bk2[:])
                    tau_sb = persist.tile([128, 1], F32, tag="tau")
                    qa.dma_start(out=tau_sb[:], in_=tau8[:])
                    del8_sb = persist.tile([128, NKT], F32, tag="del8")
                    qa.dma_start(out=del8_sb[:], in_=delta8[:])
            for c in range(NHC):
                for name, w in (("q", wq_t), ("k", wk_t)):
                    queues[(c + 1) % 2].dma_start(
                        out=w_sb[name][c][:, 128:256],
                        in_=w[c * 128 : (c + 1) * 128, 128:256].bitcast(F32R),
                    )
            wo_sb = []
            for c in range(2):
                t = persist.tile([128, H], F32R, tag=f"wo{c}", name=f"wo{c}")
                nc.scalar.dma_start(out=t[:], in_=wo_t[c * 128 : (c + 1) * 128, :].bitcast(F32R))
                wo_sb.append(t)
            vones_f = persist.tile([128, HPC], F32, tag="vones_f")
            nc.vector.memset(vones_f[:], 1.0)

            # ---- phase A: hp0 projections only (hp1's run inside phase B's
            # keepalive slots, since units are ordered hp-outer) -------------
            q_sb = [persist.tile([128, L], F32R, tag=f"q{hp}", name=f"q{hp}") for hp in range(2)]
            k_sb = [persist.tile([128, L], F32R, tag=f"k{hp}", name=f"k{hp}") for hp in range(2)]
            for dst, wname, bias in ((q_sb, "q", bq_sb), (k_sb, "k", bk_sb)):
                ps2 = [
                    sc_ps.tile(
                        [128, 1024], F32, tag="sc", name=f"ps_proj{half}", bufs=2
                    )
                    for half in range(2)
                ]
                for c in range(NHC):
                    # one stationary load serves all 4 N=512 matmuls
                    for half in range(2):
                        for s2 in range(2):
                            nc.tensor.matmul(
                                ps2[half][:, s2 * 512 : (s2 + 1) * 512],
                                w_sb[wname][c][:, 0:128],
                                hs_sb[c][:, half * 1024 + s2 * 512 : half * 1024 + (s2 + 1) * 512],
                                start=(c == 0),
                                stop=(c == NHC - 1),
                            )
                for half in range(2):
                    nc.vector.tensor_scalar_add(
                        dst[0][:, half * 1024 : half * 1024 + 1024],
                        ps2[half][:],
                        bias[:, 0:1],
                    )

            # v: per k-tile [128, 4*65]; head h cols h*65..h*65+63, col h*65+64 = 1
            v_sb = [persist.tile([128, HPC * 65], F32R, tag=f"v{kt}", name=f"v{kt}") for kt in range(NKT)]
            for kt in range(NKT):
                ps = ctx_ps.tile([128, FPC], F32, tag="ctx2", name="ps_vproj", bufs=2)
                for c in range(NHC):
                    nc.tensor.matmul(
                        ps[:],
                        hs_sb[c][:, kt * 128 : (kt + 1) * 128],
                        w_sb["v"][c][:],
                        start=(c == 0),
                        stop=(c == NHC - 1),
                    )
                v_view = v_sb[kt][:].rearrange("p (h w) -> p h w", h=HPC)
                nc.vector.tensor_copy(
                    v_view[:, :, 0:HD],
                    ps[:].rearrange("p (h w) -> p h w", h=HPC),
                )
                nc.vector.tensor_copy(v_view[:, :, HD : HD + 1].squeeze(), vones_f[:])

            # ---- phase B (+ hp1 projections and out-projection interleaved)
            # Units ordered hp-outer: u = hp*4 + qq. Each iter's "keepalive
            # slot" runs one dense K=128 matmul from the job list (hp1 q/k
            # projection sub-blocks during units 0-3, single-hp out-projection
            # chunks after), keeping the HAM activity monitor above its
            # throttle-hold threshold while turning the slack under the
            # ACT-bound exp stream into useful work. Out partials are stored
            # per head-pair (host sums them), so every chunk is one matmul.
            # ctx is stored per (hp, quarter) so out-proj reads never alias
            # the current unit's normalize writes, and normalize reciprocal/
            # mul are deferred into the next unit so their DRAM-bounce
            # latency never blocks the in-order DVE queue.
            ctx_sbq = [
                [
                    persist.tile([128, 512], F32R, tag=f"ctx{hp}q{qq}", name=f"ctx{hp}q{qq}")
                    for qq in range(4)
                ]
                for hp in range(2)
            ]

            class ProjJob:
                """One [128,512] sub-block of the hp1 q/k projection."""

                elig = (0, 0)

                def __init__(self, wname, dst, bias_col, blk):
                    self.wname, self.dst, self.bias_col, self.blk = wname, dst, bias_col, blk
                    self.c = 0
                    self.pso = None

                def step(self):
                    if self.pso is None:
                        self.pso = ctx_ps.tile([128, 512], F32, tag="op", name="ps_j", bufs=2)
                    nc.tensor.matmul(
                        self.pso[:],
                        w_sb[self.wname][self.c][:, 128:256],
                        hs_sb[self.c][:, self.blk * 512 : (self.blk + 1) * 512],
                        start=(self.c == 0),
                        stop=(self.c == NHC - 1),
                    )
                    self.c += 1
                    if self.c == NHC:
                        nc.vector.tensor_scalar_add(
                            self.dst[:, self.blk * 512 : (self.blk + 1) * 512],
                            self.pso[:],
                            self.bias_col,
                        )
                        return True
                    return False

            class OpJob:
                """One out-projection chunk for one head-pair:
                out<hp>[lt*128:+128, nch*512:+512] = ctx_hp.T @ Wo_hp."""

                def __init__(self, hp, lt, nch):
                    self.hp, self.lt, self.nch = hp, lt, nch
                    qq = lt // 4
                    self.elig = (hp * 4 + qq + 1, 8)

                def step(self):
                    pso = ctx_ps.tile([128, 512], F32, tag="op", name="ps_o", bufs=2)
                    qq, li = divmod(self.lt, 4)
                    nc.tensor.matmul(
                        pso[:],
                        ctx_sbq[self.hp][qq][:, li * 128 : (li + 1) * 128],
                        wo_sb[self.hp][:, self.nch * 512 : (self.nch + 1) * 512],
                        start=True,
                        stop=True,
                    )
                    o_sb = work.tile([128, 512], F32, tag="ostage", name="o_sb", bufs=3)
                    nc.vector.tensor_copy(o_sb[:], pso[:])
                    nc.sync.dma_start(
                        out=outs_d[self.hp][self.lt * 128 : (self.lt + 1) * 128, self.nch * 512 : (self.nch + 1) * 512],
                        in_=o_sb[:],
                    )
                    return True

            jobs = []
            for dst, wname, bias in ((q_sb, "q", bq_sb), (k_sb, "k", bk_sb)):
                for blk in range(4):
                    jobs.append(ProjJob(wname, dst[1], bias[:, 1:2], blk))
            for hp in range(2):
                for qq in range(4):
                    for li in range(4):
                        for nch in range(2):
                            jobs.append(OpJob(hp, 4 * qq + li, nch))
            cur_job = [None]

            def pump(u, kt):
                """Emit at most one keepalive-slot PE matmul."""
                if cur_job[0] is None:
                    for j in jobs:
                        if j.elig <= (u, kt):
                            cur_job[0] = j
                            jobs.remove(j)
                            break
                    else:
                        return
                if cur_job[0].step():
                    cur_job[0] = None

            deferred_norm = [[]]  # finishers from the previous unit

            for hp in range(2):
                for qq in range(4):
                    u = hp * 4 + qq
                    qoff = qq * 512
                    ctx2 = [
                        ctx_ps.tile([65, 512], F32, tag="ctx2", name=f"ctx_u{u}h{hh}", bufs=2)
                        for hh in range(2)
                    ]

                    def emit_ctx(kt0, e, hp=hp, ctx2=ctx2):
                        for hh in range(2):
                            h = hp * 2 + hh
                            nc.tensor.matmul(
                                ctx2[hh][:],
                                v_sb[kt0][:, h * 65 : (h + 1) * 65],
                                e[:, hh * 512 : (hh + 1) * 512],
                                start=(kt0 == 0),
                                stop=(kt0 == NKT - 1),
                            )

                    prev = None  # (kt, e)
                    for kt in range(NKT):
                        psS = sc_ps.tile([128, 1024], F32, tag="sc", name="ps_s", bufs=2)
                        for hh in range(2):
                            nc.tensor.matmul(
                                psS[:, hh * 512 : (hh + 1) * 512],
                                k_sb[hp][hh * HD : (hh + 1) * HD, kt * 128 : (kt + 1) * 128],
                                q_sb[hp][hh * HD : (hh + 1) * HD, qoff : qoff + 512],
                                start=True,
                                stop=True,
                            )
                        if prev is not None:
                            emit_ctx(*prev)
                        pump(u, kt)
                        if kt in (4, 5) and deferred_norm[0]:
                            deferred_norm[0].pop(0)()  # prev unit recip+mul
                        e_t = work.tile([128, 1024], F32R, tag="e", name="e_t", bufs=2)
                        nc.scalar.activation(
                            e_t[:],
                            psS[:],
                            mybir.ActivationFunctionType.Exp,
                            bias=del8_sb[:, kt : kt + 1],
                            scale=tau_sb[:],
                        )
                        prev = (kt, e_t)
                    emit_ctx(*prev)

                    # normalize ctx[0:64] / ctx[64]: drain PSUM -> SBUF now
                    # (frees accumulator banks) and launch the denominator
                    # DRAM-bounce broadcasts; the dividing reciprocal+mul are
                    # deferred into the next unit's k-loop so the bounce
                    # latency is hidden.
                    finishers = []
                    for hh in range(2):
                        raw = work.tile([65, 512], F32R, tag="raw", name=f"raw{hh}", bufs=4)
                        nc.vector.tensor_copy(raw[:], ctx2[hh][:])
                        d_dram = dscratch.tile([1, 512], F32, tag="ddram", name="d_dram")
                        nc.scalar.dma_start(out=d_dram[:], in_=raw[64:65, :].bitcast(F32))
                        d_bc = work.tile([64, 512], F32, tag="dbc", name="d_bc", bufs=4)
                        nc.scalar.dma_start(
                            out=d_bc[:],
                            in_=d_dram[0:1, :].to_broadcast([64, 512]),
                        )

                        def fin(raw=raw, d_bc=d_bc, hp=hp, qq=qq, hh=hh):
                            r_sb = work.tile([64, 512], F32, tag="r", name="r_sb", bufs=2)
                            nc.vector.reciprocal_approx_fast(r_sb[:], d_bc[:])
                            nc.vector.tensor_mul(
                                ctx_sbq[hp][qq][hh * HD : (hh + 1) * HD, :],
                                raw[0:64, :],
                                r_sb[:],
                            )

                        finishers.append(fin)
                    deferred_norm[0] = finishers

            # drain: last unit's normalize finishers, then remaining jobs
            for fin in deferred_norm[0]:
                fin()
            deferred_norm[0] = []
            while jobs or cur_job[0] is not None:
                pump(99, 99)

    nc.compile()
    return nc


def _get_nc():
    if "nc" not in _NC_CACHE:
        _NC_CACHE["nc"] = _build_kernel()
    return _NC_CACHE["nc"]


def _make_in_maps(hidden_states, tau, delta, Wq, Wk, Wv, Wo, bq, bk):
    in_maps = []
    for c in range(NCORES):
        b, hg = divmod(c, HPC)
        fs = slice(hg * FPC, (hg + 1) * FPC)
        in_maps.append(
            {
                "hs_t": np.ascontiguousarray(hidden_states[b].T),
                "wq_t": np.ascontiguousarray(Wq[fs, :].T),
                "wk_t": np.ascontiguousarray(Wk[fs, :].T),
                "wv_t": np.ascontiguousarray(Wv[fs, :].T),
                "wo_t": np.ascontiguousarray(Wo[:, fs].T),
                "bq2": np.ascontiguousarray(bq[fs].reshape(2, 128).T),
                "bk2": np.ascontiguousarray(bk[fs].reshape(2, 128).T),
                "tau8": np.full((128, 1), tau[b, 0] / 8.0, dtype=np.float32),
                "delta8": np.ascontiguousarray((delta[b] / 8.0).reshape(NKT, 128).T),
            }
        )
    return in_maps


def kernel(hidden_states, tau, delta, Wq, bq, Wk, bk, Wv, bv, Wo, bo, _trace=False):
    hidden_states = np.asarray(hidden_states, dtype=np.float32)
    tau = np.asarray(tau, dtype=np.float32)
    delta = np.asarray(delta, dtype=np.float32)
    Wq = np.asarray(Wq, dtype=np.float32)
    Wk = np.asarray(Wk, dtype=np.float32)
    Wv = np.asarray(Wv, dtype=np.float32)
    Wo = np.asarray(Wo, dtype=np.float32)
    bq = np.asarray(bq, dtype=np.float32)
    bk = np.asarray(bk, dtype=np.float32)
    bv = np.asarray(bv, dtype=np.float32)
    bo = np.asarray(bo, dtype=np.float32)

    nc = _get_nc()
    in_maps = _make_in_maps(hidden_states, tau, delta, Wq, Wk, Wv, Wo, bq, bk)
    res = run_bass_kernel_spmd(nc, in_maps, list(range(NCORES)), trace=_trace)

    out = np.zeros((B, L, H), dtype=np.float32)
    for c in range(NCORES):
        out[c // HPC] += res.results[c]["out0"]
        out[c // HPC] += res.results[c]["out1"]
    # v/out-proj biases commute through softmax-normalized attention exactly
    out += bv @ Wo.T + bo
    if _trace:
        kernel._last_exec_time_ns = res.exec_time_ns
        kernel._last_profile_json = res.profile_json
    return out
